# revision 1
# baseline (speedup 1.0000x reference)
"""Trainium2 Bass kernel for pointer-generator final-distribution (scatter_memory).

out[r, v] = p_gens[r] * vocab_ds[r, v]  (+ (1-p_gens[r])*attns[r, l_win]  at
v == sources[l, b(r)], duplicate source ids resolved last-occurrence-wins)

Strategy (8 NeuronCores, SPMD), measured ~270-310us HW (DMA-saturated):
  - Shard by batch column: core k owns b in {4k..4k+3}, all T decoder steps
    (rows r = t*B + b). Host pre-gathers rows b-major so device DMAs are
    contiguous; two 128-row groups per core (2 b's x 64 t each).
  - Stream out = p * vocab through SBUF ([128, 4096] tiles, ACT does the
    per-partition scale). Loads go on the sync HWDGE queue, stores on the
    scalar HWDGE queue — separate FIFOs avoid head-of-line blocking.
  - The scatter is applied in SBUF before the store via a compact one-hot
    matmul on the (otherwise idle) PE: for each 512-wide subtile, host bakes
    a [3K, 128] block of update values (update k x row, block-diagonal over
    the two b's) and relative target columns ck; device builds the one-hot
    [3K, 512] with is_equal(iota, ck) on DVE and PE computes
    proj = vals.T @ onehot into PSUM; DVE adds it into the streamed tile.
  - vals are an exact 3-way bf16 mantissa split (hi/mid/lo chunks stacked
    along the contraction dim): products are 1.0*chunk, f32 PSUM
    accumulation reconstructs the f32 value bit-exactly at single-pass
    bf16 matmul speed (f32 matmul would be 4 cycles/col; bf16 is 1).
    Result matches the jax reference bit-for-bit.
"""

import numpy as np

N_CORES = 8
WIN = 4096
SUB = 512


def _host_prep(vocab_ds, attns, p_gens, sources, T):
    f32 = np.float32
    vocab_ds = np.ascontiguousarray(vocab_ds, dtype=f32)
    attns = np.ascontiguousarray(attns, dtype=f32)
    p_gens = np.ascontiguousarray(p_gens, dtype=f32)
    src = np.asarray(sources).astype(np.int64)
    rows, V = vocab_ds.shape
    L, B = src.shape
    assert rows == T * B

    ag = (f32(1.0) - p_gens) * attns  # gated copy dist, [rows, L]

    # winners per batch column: duplicate source ids -> last occurrence wins
    wins = []
    for b in range(B):
        d = {}
        col = src[:, b]
        for l in range(L):
            d[int(col[l])] = l
        cols = np.fromiter(d.keys(), dtype=np.int64)
        ls = np.fromiter(d.values(), dtype=np.int64)
        o = np.argsort(cols)
        wins.append((cols[o], ls[o]))

    NW = (V + WIN - 1) // WIN
    # subtile geometry, shared by all cores/groups
    sub_geom = []  # (w, s, c0_abs, width)
    for w in range(NW):
        ww = min(WIN, V - w * WIN)
        for s in range((ww + SUB - 1) // SUB):
            sub_geom.append((w, s, w * WIN + s * SUB, min(SUB, ww - s * SUB)))
    NS = len(sub_geom)
    sub_of = {}
    for i, (w, s, c0, wd) in enumerate(sub_geom):
        sub_of[(w, s)] = i

    BPC = B // N_CORES  # 4
    G = BPC // 2        # 2 groups of 2 b's

    # bucket updates per (core, g, subtile)
    upd = [[[[] for _ in range(NS)] for _ in range(G)] for _ in range(N_CORES)]
    for core in range(N_CORES):
        for g in range(G):
            for half in range(2):
                b = core * BPC + g * 2 + half
                cols, ls = wins[b]
                for c, l in zip(cols.tolist(), ls.tolist()):
                    w = c // WIN
                    s = (c - w * WIN) // SUB
                    i = sub_of[(w, s)]
                    upd[core][g][i].append((half, c, l))

    # uniform-per-(g, subtile) K across cores
    K_ws = [[max(len(upd[core][g][i]) for core in range(N_CORES)) for i in range(NS)]
            for g in range(G)]
    KMAX = [max(K_ws[g]) if NS else 0 for g in range(G)]
    assert all(3 * k <= 128 for g in range(G) for k in K_ws[g]), \
        "subtile update count exceeds the 128-partition 3-way-split budget"
    # per-(g, window): first subtile index, #subtiles, max 3K (partition
    # extent of the JIT vals load for that window; 3x for hi/mid/lo split)
    win_info = []
    for g in range(G):
        wi = []
        for w in range(NW):
            idxs = [i for i, (w2, s2, _, _) in enumerate(sub_geom) if w2 == w]
            i0, nsub = idxs[0], len(idxs)
            kw = max(3 * K_ws[g][i] for i in idxs)
            wi.append((i0, nsub, kw))
        win_info.append(wi)

    # per-core device inputs
    in_maps = []
    for core in range(N_CORES):
        m = {}
        for g in range(G):
            row_idx = []
            for half in range(2):
                b = core * BPC + g * 2 + half
                row_idx.extend(t * B + b for t in range(T))
            row_idx = np.asarray(row_idx)
            m[f"vocab{g}"] = vocab_ds[row_idx]
            m[f"pgen{g}"] = p_gens[row_idx]
            import ml_dtypes
            bf16 = ml_dtypes.bfloat16
            vals = np.zeros((128, NS * 128), dtype=f32)
            ck = np.full((128, NS), -1.0, dtype=f32)
            for i in range(NS):
                w, s, c0, wd = sub_geom[i]
                for k, (half, c, l) in enumerate(upd[core][g][i]):
                    # rows of this b occupy partitions half*T .. half*T+T
                    r0 = half * T
                    vals[k, i * 128 + r0: i * 128 + r0 + T] = ag[row_idx[r0: r0 + T], l]
                    kw = K_ws[g][i]
                    ck[k, i] = f32(c - c0)
                    ck[kw + k, i] = f32(c - c0)
                    ck[2 * kw + k, i] = f32(c - c0)
            # exact 3-way bf16 split: val = hi + mid + lo, each chunk
            # bf16-representable; accumulating the three 1.0*chunk products
            # in f32 PSUM reconstructs val bit-exactly.
            u = vals.view(np.uint32)
            hi = (u & np.uint32(0xFFFF0000)).view(f32)
            r1 = vals - hi
            mid = (r1.view(np.uint32) & np.uint32(0xFFFF0000)).view(f32)
            lo = r1 - mid
            vals3 = np.zeros((128, NS * 128), dtype=bf16)
            for i in range(NS):
                kw = K_ws[g][i]
                blk = slice(i * 128, (i + 1) * 128)
                vals3[0:kw, blk] = hi[0:kw, blk].astype(bf16)
                vals3[kw:2 * kw, blk] = mid[0:kw, blk].astype(bf16)
                vals3[2 * kw:3 * kw, blk] = lo[0:kw, blk].astype(bf16)
            m[f"vals{g}"] = vals3
            m[f"ck{g}"] = ck
        m["iota"] = np.broadcast_to(
            np.arange(SUB, dtype=f32), (128, SUB)).copy()
        in_maps.append(m)

    meta = dict(V=V, T=T, B=B, NW=NW, NS=NS, G=G, sub_geom=sub_geom,
                sub_of=sub_of, K_ws=K_ws, KMAX=KMAX, BPC=BPC,
                win_info=win_info)
    return in_maps, meta


def _build_nc(meta):
    from concourse import bacc, mybir

    V, NW, NS, G = meta["V"], meta["NW"], meta["NS"], meta["G"]
    sub_geom, K_ws, KMAX = meta["sub_geom"], meta["K_ws"], meta["KMAX"]
    f32 = mybir.dt.float32

    bf16 = mybir.dt.bfloat16
    nc = bacc.Bacc(None, target_bir_lowering=False, debug=False)
    vocab = [nc.declare_dram_parameter(f"vocab{g}", [128, V], f32, isOutput=False)
             for g in range(G)]
    pgen = [nc.declare_dram_parameter(f"pgen{g}", [128, 1], f32, isOutput=False)
            for g in range(G)]
    vals = [nc.declare_dram_parameter(f"vals{g}", [128, NS * 128], bf16, isOutput=False)
            for g in range(G)]
    ck = [nc.declare_dram_parameter(f"ck{g}", [128, NS], f32, isOutput=False)
          for g in range(G)]
    iota = nc.declare_dram_parameter("iota", [128, SUB], f32, isOutput=False)
    out = [nc.declare_dram_parameter(f"out{g}", [128, V], f32, isOutput=True)
           for g in range(G)]

    from concourse.tile import TileContext

    win_info = meta["win_info"]
    with TileContext(nc) as tc:
        with tc.tile_pool(name="io", bufs=6) as io_pool, \
             tc.tile_pool(name="small", bufs=1) as small, \
             tc.tile_pool(name="oh", bufs=8) as oh_pool, \
             tc.tile_pool(name="psum", bufs=8, space="PSUM") as psum_pool:

            iota_t = small.tile([128, SUB], f32)
            nc.sync.dma_start(out=iota_t[:], in_=iota[:])

            for g in range(G):
                p_t = small.tile([128, 1], f32, tag=f"p{g}")
                nc.sync.dma_start(out=p_t[:], in_=pgen[g][:])
                ck_t = small.tile([128, NS], f32, tag=f"ck{g}")
                nc.sync.dma_start(out=ck_t[:], in_=ck[g][:])
                kmax3 = max(wi[2] for wi in win_info[g])
                vals_t = small.tile([128, NS * 128], bf16, tag=f"vals{g}")
                nc.sync.dma_start(out=vals_t[:kmax3, :], in_=vals[g][:kmax3, :])

                for w in range(NW):
                    c0w = w * WIN
                    ww = min(WIN, V - c0w)
                    i0, nsub, kw = win_info[g][w]
                    t = io_pool.tile([128, WIN], f32, tag="io")
                    nc.sync.dma_start(out=t[:, :ww], in_=vocab[g][:, c0w:c0w + ww])
                    nc.scalar.activation(
                        t[:, :ww], t[:, :ww],
                        mybir.ActivationFunctionType.Copy, scale=p_t[:, :1])
                    for s in range(nsub):
                        i = i0 + s
                        K = K_ws[g][i]
                        if K == 0:
                            continue
                        _, _, c0, wd = sub_geom[i]
                        K3 = 3 * K
                        oh = oh_pool.tile([128, SUB], bf16, tag="oh")
                        nc.vector.tensor_scalar(
                            out=oh[:K3, :wd], in0=iota_t[:K3, :wd],
                            scalar1=ck_t[:K3, i:i + 1], scalar2=None,
                            op0=mybir.AluOpType.is_equal)
                        ps = psum_pool.tile([128, SUB], f32, tag="ps")
                        nc.tensor.matmul(
                            out=ps[:, :wd],
                            lhsT=vals_t[:K3, i * 128:(i + 1) * 128],
                            rhs=oh[:K3, :wd],
                            start=True, stop=True)
                        lo = c0 - c0w
                        nc.vector.tensor_add(
                            out=t[:, lo:lo + wd], in0=t[:, lo:lo + wd],
                            in1=ps[:, :wd])
                    nc.scalar.dma_start(out=out[g][:, c0w:c0w + ww], in_=t[:, :ww])
    nc.finalize()
    return nc


def kernel(vocab_ds, attns, p_gens, sources, decoder_batch_len):
    T = int(decoder_batch_len)
    in_maps, meta = _host_prep(vocab_ds, attns, p_gens, sources, T)
    nc = _build_nc(meta)

    from concourse.bass_utils import run_bass_kernel_spmd
    res = run_bass_kernel_spmd(nc, in_maps, list(range(N_CORES)))

    rows, V = np.asarray(vocab_ds).shape
    B, BPC, G = meta["B"], meta["BPC"], meta["G"]
    full = np.empty((rows, V), dtype=np.float32)
    for core in range(N_CORES):
        for g in range(G):
            blk = res.results[core][f"out{g}"]
            for half in range(2):
                b = core * BPC + g * 2 + half
                full[b::B] = blk[half * T:(half + 1) * T]
    return full



# revision 2
# speedup vs baseline: 1.2368x; 1.2368x over previous
"""Trainium2 Bass kernel for pointer-generator final-distribution (scatter_memory).

out[r, v] = p_gens[r] * vocab_ds[r, v]  (+ (1-p_gens[r])*attns[r, l_win]  at
v == sources[l, b(r)], duplicate source ids resolved last-occurrence-wins)

Strategy (8 NeuronCores, SPMD), bf16 streaming:
  - The rel-err gate is 2e-2 and every term is non-negative (no cancellation),
    so the whole pipeline runs in bf16 (worst-case stacked rounding ~6e-3).
    Host downcasts vocab_ds to bf16, the device reads/writes bf16, and the
    host upconverts the result: HBM traffic per core drops from 103 MB (f32)
    to ~53 MB, i.e. a ~150 us DMA floor instead of ~290 us.
  - Shard by batch column: core k owns b in {4k..4k+3}; two 128-row groups
    per core (2 b's x 64 t each), rows gathered b-major on host so device
    DMAs are contiguous. Loads stream on the sync HWDGE ring, stores on the
    scalar HWDGE ring (separate FIFOs avoid head-of-line blocking).
  - Per 512-wide subtile the PE does all the math into PSUM:
      mm1: diag(p) @ vocab_tile          (start=True; per-row scale)
      mm2: vals.T @ onehot  (accumulate) (scatter of (1-p)*attn values)
    then a single PSUM->SBUF bf16 copy (alternating ACT/DVE) produces the
    output tile. vals/onehot blocks live in 32-partition slots (4 subtiles
    per 128-partition "page") so matmul base partitions stay 32-aligned;
    mm2 passes tile_position=(slot*32, 0) explicitly.
  - One-hots are built on device: one DVE is_equal(iota, ck) per page
    (4 subtiles at once), just-in-time before the window that consumes it.
"""

import numpy as np
import ml_dtypes

N_CORES = 8
WIN = 8192          # streaming window (columns) per tile
SUB = 512           # matmul/psum subtile width (one PSUM bank in f32)
SLOT = 32           # partition rows per scatter block (32-aligned for PE)
BF16 = ml_dtypes.bfloat16


def _host_prep(vocab_ds, attns, p_gens, sources, T):
    f32 = np.float32
    vocab_ds = np.ascontiguousarray(np.asarray(vocab_ds), dtype=f32)
    attns = np.ascontiguousarray(np.asarray(attns), dtype=f32)
    p_gens = np.ascontiguousarray(np.asarray(p_gens), dtype=f32).reshape(-1, 1)
    src = np.asarray(sources).astype(np.int64)
    rows, V = vocab_ds.shape
    L, B = src.shape
    assert rows == T * B
    BPC = B // N_CORES          # batch cols per core (4)
    G = BPC // 2                # groups of 2 b's -> 128 partitions (2)
    H = T                       # rows per half-group
    assert 2 * H == 128 and B % N_CORES == 0 and BPC % 2 == 0

    ag = (f32(1.0) - p_gens) * attns            # gated copy dist, f32
    ag_bf = ag.astype(BF16)
    # per-b [L, T] contiguous views of ag for fast row baking
    agT = [np.ascontiguousarray(ag_bf[b::B, :].T) for b in range(B)]

    # winners per batch column: duplicate source ids -> last occurrence wins
    wins = []
    for b in range(B):
        d = {}
        col = src[:, b]
        for l in range(L):
            d[int(col[l])] = l
        wins.append(sorted(d.items()))

    # subtile geometry: windows of WIN cols, subtiles of SUB cols
    SPW = WIN // SUB
    sub_list = []               # (c0, wd)
    for w0 in range(0, V, WIN):
        ww = min(WIN, V - w0)
        for s0 in range(0, ww, SUB):
            sub_list.append((w0 + s0, min(SUB, ww - s0)))
    NS = len(sub_list)

    def sub_of(c):
        return (c // WIN) * SPW + (c % WIN) // SUB

    # updates[core][g][i] = list of (half, c, l)
    upd = [[[[] for _ in range(NS)] for _ in range(G)] for _ in range(N_CORES)]
    for core in range(N_CORES):
        for g in range(G):
            for half in range(2):
                b = core * BPC + g * 2 + half
                for c, l in wins[b]:
                    upd[core][g][sub_of(c)].append((half, c, l))

    # uniform block layout across cores: K_i = max update count per subtile,
    # split into ceil(K/SLOT) blocks of SLOT rows, packed 4 blocks per page
    K = [[max(len(upd[core][g][i]) for core in range(N_CORES))
          for i in range(NS)] for g in range(G)]
    blocks = []                 # per g: per i: list of (page, slot, k0)
    NP = []
    for g in range(G):
        binfo = []
        page, slot = 0, 0
        for i in range(NS):
            nblk = max(1, -(-K[g][i] // SLOT))
            bl = []
            for j in range(nblk):
                bl.append((page, slot, j * SLOT))
                slot += 1
                if slot == 128 // SLOT:
                    slot = 0
                    page += 1
            binfo.append(bl)
        blocks.append(binfo)
        NP.append(page + (1 if slot else 0))

    # per-core device inputs
    in_maps = []
    iota = np.broadcast_to(np.arange(SUB, dtype=f32), (128, SUB)).copy()
    vb = vocab_ds.astype(BF16).reshape(T, B, V)
    p_col = p_gens.reshape(T, B)
    for core in range(N_CORES):
        m = {"iota": iota}
        for g in range(G):
            b0 = core * BPC + 2 * g
            m[f"vocab{g}"] = np.ascontiguousarray(
                np.concatenate([vb[:, b0], vb[:, b0 + 1]], axis=0))
            dg = np.zeros((128, 128), dtype=BF16)
            pv = np.concatenate([p_col[:, b0], p_col[:, b0 + 1]])
            dg[np.arange(128), np.arange(128)] = pv.astype(BF16)
            m[f"diag{g}"] = dg
            ck = np.full((128, NP[g]), -1.0, dtype=f32)
            va = np.zeros((128, NP[g] * 128), dtype=BF16)
            for i in range(NS):
                c0, wd = sub_list[i]
                ups = upd[core][g][i]
                for (page, slot, k0) in blocks[g][i]:
                    for j, (half, c, l) in enumerate(ups[k0:k0 + SLOT]):
                        r = slot * SLOT + j
                        ck[r, page] = f32(c - c0)
                        b = core * BPC + 2 * g + half
                        va[r, page * 128 + half * H:
                           page * 128 + (half + 1) * H] = agT[b][l]
            m[f"ck{g}"] = ck
            m[f"vals{g}"] = va
        in_maps.append(m)

    meta = dict(V=V, T=T, B=B, L=L, BPC=BPC, G=G, NS=NS, NP=NP,
                sub_list=sub_list, blocks=blocks, SPW=SPW)
    return in_maps, meta


def _build_nc(meta):
    from concourse import bacc, mybir
    from concourse.tile import TileContext

    V, G, NS, NP = meta["V"], meta["G"], meta["NS"], meta["NP"]
    sub_list, blocks, SPW = meta["sub_list"], meta["blocks"], meta["SPW"]
    f32 = mybir.dt.float32
    bf16 = mybir.dt.bfloat16

    nc = bacc.Bacc(None, target_bir_lowering=False, debug=False)
    vocab = [nc.declare_dram_parameter(f"vocab{g}", [128, V], bf16, isOutput=False)
             for g in range(G)]
    diag = [nc.declare_dram_parameter(f"diag{g}", [128, 128], bf16, isOutput=False)
            for g in range(G)]
    vals = [nc.declare_dram_parameter(f"vals{g}", [128, NP[g] * 128], bf16, isOutput=False)
            for g in range(G)]
    ck = [nc.declare_dram_parameter(f"ck{g}", [128, NP[g]], f32, isOutput=False)
          for g in range(G)]
    iota = nc.declare_dram_parameter("iota", [128, SUB], f32, isOutput=False)
    out = [nc.declare_dram_parameter(f"out{g}", [128, V], bf16, isOutput=True)
           for g in range(G)]

    with TileContext(nc) as tc:
        with tc.tile_pool(name="ld", bufs=4) as ld_pool, \
             tc.tile_pool(name="st", bufs=4) as st_pool, \
             tc.tile_pool(name="oh", bufs=8) as oh_pool, \
             tc.tile_pool(name="small", bufs=1) as small, \
             tc.tile_pool(name="psum", bufs=8, space="PSUM") as psum_pool:

            iota_t = small.tile([128, SUB], f32, tag="iota")
            nc.scalar.dma_start(out=iota_t[:], in_=iota[:])
            diag_t, vals_t, ck_t = [], [], []
            for g in range(G):
                d = small.tile([128, 128], bf16, tag=f"diag{g}")
                nc.scalar.dma_start(out=d[:], in_=diag[g][:])
                v = small.tile([128, NP[g] * 128], bf16, tag=f"vals{g}")
                nc.scalar.dma_start(out=v[:], in_=vals[g][:])
                c = small.tile([128, NP[g]], f32, tag=f"ck{g}")
                nc.scalar.dma_start(out=c[:], in_=ck[g][:])
                diag_t.append(d)
                vals_t.append(v)
                ck_t.append(c)

            cp = 0
            for g in range(G):
                page_tiles = {}
                for w0 in range(0, V, WIN):
                    ww = min(WIN, V - w0)
                    ti = ld_pool.tile([128, WIN], bf16, tag="ld")
                    nc.sync.dma_start(out=ti[:, :ww], in_=vocab[g][:, w0:w0 + ww])
                    to = st_pool.tile([128, WIN], bf16, tag="st")
                    for s0 in range(0, ww, SUB):
                        i = (w0 // WIN) * SPW + s0 // SUB
                        c0, wd = sub_list[i]
                        bl = blocks[g][i]
                        ps = psum_pool.tile([128, SUB], f32, tag="ps")
                        nc.tensor.matmul(
                            out=ps[:, :wd], lhsT=diag_t[g][:, :],
                            rhs=ti[:, s0:s0 + wd], start=True, stop=False)
                        for bi, (page, slot, k0) in enumerate(bl):
                            if page not in page_tiles:
                                ohp = oh_pool.tile([128, SUB], bf16, tag="oh")
                                nc.vector.tensor_scalar(
                                    out=ohp[:, :], in0=iota_t[:, :],
                                    scalar1=ck_t[g][:, page:page + 1],
                                    scalar2=None,
                                    op0=mybir.AluOpType.is_equal)
                                page_tiles[page] = ohp
                            p0 = slot * SLOT
                            nc.tensor.matmul(
                                out=ps[:, :wd],
                                lhsT=vals_t[g][p0:p0 + SLOT,
                                               page * 128:(page + 1) * 128],
                                rhs=page_tiles[page][p0:p0 + SLOT, :wd],
                                tile_position=(p0, 0),
                                start=False, stop=(bi == len(bl) - 1))
                        if cp % 2 == 0:
                            nc.scalar.activation(
                                to[:, s0:s0 + wd], ps[:, :wd],
                                mybir.ActivationFunctionType.Copy)
                        else:
                            nc.vector.tensor_scalar_add(
                                out=to[:, s0:s0 + wd], in0=ps[:, :wd],
                                scalar1=0.0)
                        cp += 1
                    nc.scalar.dma_start(out=out[g][:, w0:w0 + ww],
                                        in_=to[:, :ww])
    nc.finalize()
    return nc


def _gather_output(results, meta):
    B, BPC, G, T, V = (meta["B"], meta["BPC"], meta["G"], meta["T"], meta["V"])
    full = np.empty((T * B, V), dtype=np.float32)
    fv = full.reshape(T, B, V)
    for core in range(N_CORES):
        for g in range(G):
            blk = np.asarray(results[core][f"out{g}"]).astype(np.float32)
            b0 = core * BPC + 2 * g
            fv[:, b0] = blk[:T]
            fv[:, b0 + 1] = blk[T:]
    return full


def kernel(vocab_ds, attns, p_gens, sources, decoder_batch_len):
    T = int(decoder_batch_len)
    in_maps, meta = _host_prep(vocab_ds, attns, p_gens, sources, T)
    nc = _build_nc(meta)

    from concourse.bass_utils import run_bass_kernel_spmd
    res = run_bass_kernel_spmd(nc, in_maps, list(range(N_CORES)))
    return _gather_output(res.results, meta)


# revision 8
# speedup vs baseline: 1.7440x; 1.4100x over previous
"""Trainium2 Bass kernel for pointer-generator final-distribution (scatter_memory).

out[r, v] = p_gens[r] * vocab_ds[r, v]  (+ (1-p_gens[r])*attns[r, l_win]  at
v == sources[l, b(r)], duplicate source ids resolved last-occurrence-wins)

Strategy (8 NeuronCores, SPMD), bf16 streaming:
  - The rel-err gate is 2e-2 and every term is non-negative (no cancellation),
    so the whole pipeline runs in bf16 (worst-case stacked rounding ~6e-3).
    Host downcasts vocab_ds to bf16, the device reads/writes bf16, and the
    host upconverts the result: HBM traffic per core drops from 103 MB (f32)
    to ~53 MB, i.e. a ~150 us DMA floor instead of ~290 us.
  - Shard by batch column: core k owns b in {4k..4k+3}; two 128-row groups
    per core (2 b's x 64 t each), rows gathered b-major on host so device
    DMAs are contiguous. Loads stream on the sync HWDGE ring, stores on the
    scalar HWDGE ring (separate FIFOs avoid head-of-line blocking).
  - Engines stream ~1 column (128 partitions) per ~850MHz cycle, so every
    full pass over the data costs ~120 us/core: the budget is ONE PE pass
    and ONE DVE pass. Per 512-wide subtile:
      PE:  psum = vals.T @ onehot      (scatter of (1-p)*attn values)
      DVE: out  = vocab * p + psum     (one fused scalar_tensor_tensor)
    vals/onehot blocks live in 32-partition slots (4 subtiles per
    128-partition "page") so matmul base partitions stay 32-aligned;
    the matmul passes tile_position=(slot*32, 0) explicitly.
  - One-hots are built on device: one DVE is_equal(iota, ck) per page
    (4 subtiles at once), just-in-time before the window that consumes it.
"""

import numpy as np
import ml_dtypes

N_CORES = 8
WIN = 8192          # streaming window (columns) per tile
SUB = 512           # matmul/psum subtile width (one PSUM bank in f32)
SLOT = 32           # partition rows per scatter block (32-aligned for PE)
BF16 = ml_dtypes.bfloat16


def _host_prep(vocab_ds, attns, p_gens, sources, T):
    f32 = np.float32
    vocab_ds = np.ascontiguousarray(np.asarray(vocab_ds), dtype=f32)
    attns = np.ascontiguousarray(np.asarray(attns), dtype=f32)
    p_gens = np.ascontiguousarray(np.asarray(p_gens), dtype=f32).reshape(-1, 1)
    src = np.asarray(sources).astype(np.int64)
    rows, V = vocab_ds.shape
    L, B = src.shape
    assert rows == T * B
    BPC = B // N_CORES          # batch cols per core (4)
    G = BPC // 2                # groups of 2 b's -> 128 partitions (2)
    H = T                       # rows per half-group
    assert 2 * H == 128 and B % N_CORES == 0 and BPC % 2 == 0

    ag = (f32(1.0) - p_gens) * attns            # gated copy dist, f32
    ag_bf = ag.astype(BF16)
    # per-b [L, T] contiguous views of ag for fast row baking
    agT = [np.ascontiguousarray(ag_bf[b::B, :].T) for b in range(B)]

    # winners per batch column: duplicate source ids -> last occurrence wins
    wins = []
    for b in range(B):
        d = {}
        col = src[:, b]
        for l in range(L):
            d[int(col[l])] = l
        wins.append(sorted(d.items()))

    # subtile geometry: windows of WIN cols, subtiles of SUB cols
    SPW = WIN // SUB
    sub_list = []               # (c0, wd)
    for w0 in range(0, V, WIN):
        ww = min(WIN, V - w0)
        for s0 in range(0, ww, SUB):
            sub_list.append((w0 + s0, min(SUB, ww - s0)))
    NS = len(sub_list)

    def sub_of(c):
        return (c // WIN) * SPW + (c % WIN) // SUB

    # updates[core][g][i] = list of (half, c, l)
    upd = [[[[] for _ in range(NS)] for _ in range(G)] for _ in range(N_CORES)]
    for core in range(N_CORES):
        for g in range(G):
            for half in range(2):
                b = core * BPC + g * 2 + half
                for c, l in wins[b]:
                    upd[core][g][sub_of(c)].append((half, c, l))

    # uniform block layout across cores: K_i = max update count per subtile,
    # split into ceil(K/SLOT) blocks of SLOT rows, packed 4 blocks per page
    K = [[max(len(upd[core][g][i]) for core in range(N_CORES))
          for i in range(NS)] for g in range(G)]
    blocks = []                 # per g: per i: list of (page, slot, k0)
    NP = []
    for g in range(G):
        binfo = []
        page, slot = 0, 0
        for i in range(NS):
            nblk = max(1, -(-K[g][i] // SLOT))
            bl = []
            for j in range(nblk):
                bl.append((page, slot, j * SLOT))
                slot += 1
                if slot == 128 // SLOT:
                    slot = 0
                    page += 1
            binfo.append(bl)
        blocks.append(binfo)
        NP.append(page + (1 if slot else 0))

    # per-core device inputs
    in_maps = []
    iota = np.broadcast_to(np.arange(SUB, dtype=f32), (128, SUB)).copy()
    vb = vocab_ds.astype(BF16).reshape(T, B, V)
    p_col = p_gens.reshape(T, B)
    for core in range(N_CORES):
        m = {"iota": iota}
        for g in range(G):
            b0 = core * BPC + 2 * g
            m[f"vocab{g}"] = np.ascontiguousarray(
                np.concatenate([vb[:, b0], vb[:, b0 + 1]], axis=0))
            m[f"pgen{g}"] = np.ascontiguousarray(
                np.concatenate([p_col[:, b0], p_col[:, b0 + 1]])
                .reshape(128, 1))
            ck = np.full((128, NP[g]), -1.0, dtype=f32)
            va = np.zeros((128, NP[g] * 128), dtype=BF16)
            for i in range(NS):
                c0, wd = sub_list[i]
                ups = upd[core][g][i]
                for (page, slot, k0) in blocks[g][i]:
                    for j, (half, c, l) in enumerate(ups[k0:k0 + SLOT]):
                        r = slot * SLOT + j
                        ck[r, page] = f32(c - c0)
                        b = core * BPC + 2 * g + half
                        va[r, page * 128 + half * H:
                           page * 128 + (half + 1) * H] = agT[b][l]
            m[f"ck{g}"] = ck
            m[f"vals{g}"] = va
        in_maps.append(m)

    meta = dict(V=V, T=T, B=B, L=L, BPC=BPC, G=G, NS=NS, NP=NP,
                sub_list=sub_list, blocks=blocks, SPW=SPW)
    return in_maps, meta


def _build_nc(meta):
    from concourse import bacc, mybir
    from concourse.tile import TileContext

    V, G, NS, NP = meta["V"], meta["G"], meta["NS"], meta["NP"]
    sub_list, blocks, SPW = meta["sub_list"], meta["blocks"], meta["SPW"]
    f32 = mybir.dt.float32
    bf16 = mybir.dt.bfloat16

    nc = bacc.Bacc(None, target_bir_lowering=False, debug=False)
    vocab = [nc.declare_dram_parameter(f"vocab{g}", [128, V], bf16, isOutput=False)
             for g in range(G)]
    pgen = [nc.declare_dram_parameter(f"pgen{g}", [128, 1], f32, isOutput=False)
            for g in range(G)]
    vals = [nc.declare_dram_parameter(f"vals{g}", [128, NP[g] * 128], bf16, isOutput=False)
            for g in range(G)]
    ck = [nc.declare_dram_parameter(f"ck{g}", [128, NP[g]], f32, isOutput=False)
          for g in range(G)]
    iota = nc.declare_dram_parameter("iota", [128, SUB], f32, isOutput=False)
    out = [nc.declare_dram_parameter(f"out{g}", [128, V], bf16, isOutput=True)
           for g in range(G)]

    with TileContext(nc) as tc:
        with tc.tile_pool(name="ld", bufs=4) as ld_pool, \
             tc.tile_pool(name="st", bufs=4) as st_pool, \
             tc.tile_pool(name="oh", bufs=8) as oh_pool, \
             tc.tile_pool(name="small", bufs=1) as small, \
             tc.tile_pool(name="psum", bufs=8, space="PSUM") as psum_pool:

            iota_t = small.tile([128, SUB], f32, tag="iota")
            nc.scalar.dma_start(out=iota_t[:], in_=iota[:])
            pgen_t, vals_t, ck_t = [], [], []
            for g in range(G):
                p = small.tile([128, 1], f32, tag=f"pgen{g}")
                nc.scalar.dma_start(out=p[:], in_=pgen[g][:])
                v = small.tile([128, NP[g] * 128], bf16, tag=f"vals{g}")
                nc.scalar.dma_start(out=v[:], in_=vals[g][:])
                c = small.tile([128, NP[g]], f32, tag=f"ck{g}")
                nc.scalar.dma_start(out=c[:], in_=ck[g][:])
                pgen_t.append(p)
                vals_t.append(v)
                ck_t.append(c)

            for g in range(G):
                page_tiles = {}
                for w0 in range(0, V, WIN):
                    ww = min(WIN, V - w0)
                    ti = ld_pool.tile([128, WIN], bf16, tag="ld")
                    nc.sync.dma_start(out=ti[:, :ww], in_=vocab[g][:, w0:w0 + ww])
                    to = st_pool.tile([128, WIN], bf16, tag="st")
                    for s0 in range(0, ww, SUB):
                        i = (w0 // WIN) * SPW + s0 // SUB
                        c0, wd = sub_list[i]
                        bl = blocks[g][i]
                        ps = psum_pool.tile([128, SUB], f32, tag="ps")
                        for bi, (page, slot, k0) in enumerate(bl):
                            if page not in page_tiles:
                                ohp = oh_pool.tile([128, SUB], bf16, tag="oh")
                                nc.vector.tensor_scalar(
                                    out=ohp[:, :], in0=iota_t[:, :],
                                    scalar1=ck_t[g][:, page:page + 1],
                                    scalar2=None,
                                    op0=mybir.AluOpType.is_equal)
                                page_tiles[page] = ohp
                            p0 = slot * SLOT
                            nc.tensor.matmul(
                                out=ps[:, :wd],
                                lhsT=vals_t[g][p0:p0 + SLOT,
                                               page * 128:(page + 1) * 128],
                                rhs=page_tiles[page][p0:p0 + SLOT, :wd],
                                tile_position=(p0, 0),
                                start=(bi == 0), stop=(bi == len(bl) - 1))
                        nc.vector.scalar_tensor_tensor(
                            out=to[:, s0:s0 + wd], in0=ti[:, s0:s0 + wd],
                            scalar=pgen_t[g][:, 0:1], in1=ps[:, :wd],
                            op0=mybir.AluOpType.mult,
                            op1=mybir.AluOpType.add)
                    nc.scalar.dma_start(out=out[g][:, w0:w0 + ww],
                                        in_=to[:, :ww])
    nc.finalize()
    return nc


def _gather_output(results, meta):
    B, BPC, G, T, V = (meta["B"], meta["BPC"], meta["G"], meta["T"], meta["V"])
    full = np.empty((T * B, V), dtype=np.float32)
    fv = full.reshape(T, B, V)
    for core in range(N_CORES):
        for g in range(G):
            blk = np.asarray(results[core][f"out{g}"]).astype(np.float32)
            b0 = core * BPC + 2 * g
            fv[:, b0] = blk[:T]
            fv[:, b0 + 1] = blk[T:]
    return full


def kernel(vocab_ds, attns, p_gens, sources, decoder_batch_len):
    T = int(decoder_batch_len)
    in_maps, meta = _host_prep(vocab_ds, attns, p_gens, sources, T)
    nc = _build_nc(meta)

    from concourse.bass_utils import run_bass_kernel_spmd
    res = run_bass_kernel_spmd(nc, in_maps, list(range(N_CORES)))
    return _gather_output(res.results, meta)
